# revision 10
# baseline (speedup 1.0000x reference)
"""Trainium2 Bass kernel for nn_MemoryBank_51135880626820 (scatter_memory).

Data-parallel over the query batch across 8 NeuronCores: the [32768, 256]
memory bank is replicated per core, each core handles 1024 query rows.

Per-core pipeline:
  - normalize memory rows, cast bf16, DMA-transpose roundtrip to build a
    resident d-major [256, 32768] bf16 bank (two [128, 32768] SBUF tiles)
  - per 128-query tile: bf16 matmul (fp32 PSUM) -> ACT evacuation to fp32
    SBUF sectors -> DVE max8/max_index screening -> prune to 16 candidates
    -> indirect-DMA gather of candidate memory rows (fp32) -> exact fp32
    re-dot on DVE -> top-8 + softmax + weighted sum -> renormalize to ||q||.

self-contained: hardcodes all shapes; builds and caches the Bass program on
first call.
"""

import sys

for _p in ("/opt/trn_rl_repo",):
    if _p not in sys.path:
        sys.path.insert(0, _p)

import numpy as np

import concourse.bass as bass
import concourse.mybir as mybir
import concourse.tile as tile
from concourse.bass import IndirectOffsetOnAxis

F32 = mybir.dt.float32
BF16 = mybir.dt.bfloat16
U32 = mybir.dt.uint32

N_CORES = 8
B = 8192
B_LOC = B // N_CORES        # 1024
M = 32768
D = 256
K = 8
NQT = B_LOC // 128          # 8 query tiles per core
SEC = 2048                  # screening sector width (f32 SBUF)
NSEC = M // SEC
CH = 2048                   # PSUM evacuation chunk (4 banks)
LCAND = NSEC * 8            # 64 raw candidates
LP = 16                     # pruned candidates (refined exactly)
RH = 8                      # refine sub-batch (SBUF-fit)
NEG = -1.0e30
SELF_MATCH = 0.9999


# --------------------------------------------------------------------------
# workarounds for this container's walrus build, which rejects more than one
# sync-wait per instruction ("Too many sync wait commands").
# --------------------------------------------------------------------------
def _install_patches():
    import json

    import bass_rust
    import concourse.bass_utils as _bu
    import concourse.bass2jax as _b2j
    import concourse.tile as tile_mod
    from concourse.tile import TileContext

    if getattr(_bu, "_mb_patched", False):
        return

    try:
        ScopedClock = tile_mod.ScopedClock
    except AttributeError:
        ScopedClock = bass_rust.ScopedClock

    def _patched_drain_and_barrier(self, tick_clock, wait_clock):
        nc = self.nc
        drain_inst = nc.sync.drain()
        wait_clock.add_sem_waits(
            drain_inst.ins, ScopedClock({None: tick_clock.global_clock})
        )
        si = drain_inst.ins.sync_info
        waits = list(si.on_wait) if si is not None and si.on_wait else []
        if len(waits) > 1:
            drain_inst.ins.sync_info = bass_rust.SyncInfo(
                on_wait=[waits[0]],
                on_update=list(si.on_update) if si.on_update else [],
            )
            for w in waits[1:]:
                nop = nc.sync.nop(nofuse=True, hint="tail_wait")
                nop.ins.sync_info = bass_rust.SyncInfo(on_wait=[w], on_update=[])
        nc.all_engine_barrier()
        assert self.sems is not None
        popped = nc._tile_sem_poison_stack.pop()
        assert popped is self._sem_poison
        nc.clear_and_free_semaphores(list(self.sems.allocated().values()))
        nc.all_engine_barrier()

    TileContext._drain_and_barrier = _patched_drain_and_barrier

    def split_multiwaits(bir_json):
        m = json.loads(bir_json)
        changed = False
        for fn in m.get("functions", []):
            for bb in fn.get("blocks", []):
                insts = bb.get("instructions", [])
                out = []
                for ins in insts:
                    si = ins.get("sync_info") or {}
                    waits = si.get("on_wait") or []
                    if len(waits) > 1:
                        changed = True
                        for kk, w in enumerate(waits[:-1]):
                            out.append({
                                "debug": ins.get("debug", 0),
                                "engine": ins["engine"],
                                "ins": [],
                                "name": f"{ins['name']}-w{kk}",
                                "opcode": "NoOp",
                                "outs": [],
                                "sync_info": {"on_update": [], "on_wait": [w]},
                                "text_hint": "split_wait",
                            })
                        si = dict(si)
                        si["on_wait"] = [waits[-1]]
                        ins = dict(ins)
                        ins["sync_info"] = si
                    out.append(ins)
                bb["instructions"] = out
        if not changed:
            return bir_json
        return json.dumps(m).encode()

    _orig_compile = _bu.compile_bir_kernel

    def _patched_compile(bir_json, tmpdir, neff_name="file.neff"):
        if isinstance(bir_json, str):
            bir_json = bir_json.encode()
        return _orig_compile(split_multiwaits(bir_json), tmpdir, neff_name)

    _bu.compile_bir_kernel = _patched_compile
    _b2j.compile_bir_kernel = _patched_compile
    _bu._mb_patched = True


# --------------------------------------------------------------------------
# per-core Bass program
# --------------------------------------------------------------------------
def _build():
    nc = bass.Bass("TRN2", target_bir_lowering=False, debug=False)
    q_in = nc.dram_tensor("q", [B_LOC, D], F32, kind="ExternalInput")
    mem_in = nc.dram_tensor("mem", [M, D], F32, kind="ExternalInput")
    out = nc.dram_tensor("out", [B_LOC, D], F32, kind="ExternalOutput")
    nm_dram = nc.dram_tensor("nm_dram", [M, D], BF16)
    aug_dram = nc.dram_tensor("aug_dram", [M, D + 4], F32)

    with tile.TileContext(nc) as tc:
        with (
            tc.tile_pool(name="resident", bufs=1) as res_pool,
            tc.tile_pool(name="mprep", bufs=2) as mprep,
            tc.tile_pool(name="sector", bufs=2) as secp,
            tc.tile_pool(name="small", bufs=2) as small,
            tc.tile_pool(name="gat", bufs=1) as gat,
            tc.tile_pool(name="psum", bufs=2, space="PSUM") as psp,
        ):
            nmT_a = res_pool.tile([128, M], BF16, tag="nmT_a")
            nmT_b = res_pool.tile([128, M], BF16, tag="nmT_b")
            qT_a = res_pool.tile([128, 128], BF16, tag="qT_a")
            qT_b = res_pool.tile([128, 128], BF16, tag="qT_b")

            # sector base indices: value s*SEC repeated 8, as f32
            base_u = res_pool.tile([128, LCAND], mybir.dt.uint32, tag="base_u")
            nc.gpsimd.iota(base_u[:], pattern=[[SEC, NSEC], [0, 8]], base=0,
                           channel_multiplier=0)
            base_f = res_pool.tile([128, LCAND], F32, tag="base_f")
            nc.vector.tensor_copy(base_f[:], base_u[:])

            # raw rows into the aug table (dram->dram, no engine work)
            nc.sync.dma_start(aug_dram[:, 0:D], mem_in[:])
            norm_all = res_pool.tile([128, M // 128], F32, tag="norm_all")

            # ---- memory prep: grouped 1024-row iterations ----
            GR = 4  # row-tiles per group
            for g in range(M // (128 * GR)):
                r0 = g * 128 * GR
                mtg = mprep.tile([128, GR * D], F32, tag="mtg")
                nc.sync.dma_start(
                    mtg[:].rearrange("p (t d) -> p t d", t=GR),
                    mem_in[r0:r0 + 128 * GR, :].rearrange("(t p) d -> p t d", p=128))
                sq = mprep.tile([128, D], F32, tag="sq")
                nrm2 = mprep.tile([128, GR], F32, tag="nrm2")
                for tl in range(GR):
                    nc.scalar.activation(sq[:], mtg[:, tl * D:(tl + 1) * D],
                                         mybir.ActivationFunctionType.Square,
                                         accum_out=nrm2[:, tl:tl + 1])
                nrm = norm_all[:, g * GR:(g + 1) * GR]
                nc.scalar.activation(nrm, nrm2[:], mybir.ActivationFunctionType.Sqrt)
                nc.vector.tensor_scalar_max(nrm, nrm, 1e-12)
                rin = mprep.tile([128, GR], F32, tag="rin")
                nc.vector.reciprocal(rin[:], nrm)
                nmbg = mprep.tile([128, GR * D], BF16, tag="nmbg")
                for tl in range(GR):
                    nc.vector.tensor_scalar(
                        out=nmbg[:, tl * D:(tl + 1) * D],
                        in0=mtg[:, tl * D:(tl + 1) * D],
                        scalar1=rin[:, tl:tl + 1], scalar2=None,
                        op0=mybir.AluOpType.mult)
                nc.sync.dma_start(
                    nm_dram[r0:r0 + 128 * GR, :].rearrange("(t p) d -> p t d", p=128),
                    nmbg[:].rearrange("p (t d) -> p t d", t=GR))

            nc.sync.dma_start(
                aug_dram[:, D:D + 1].rearrange("(t p) c -> p (t c)", p=128),
                norm_all[:])

            for c in range(M // 2048):
                nc.sync.dma_start(
                    nmT_a[:, c * 2048 : (c + 1) * 2048],
                    nm_dram[c * 2048 : (c + 1) * 2048, 0:128], transpose=True)
                nc.sync.dma_start(
                    nmT_b[:, c * 2048 : (c + 1) * 2048],
                    nm_dram[c * 2048 : (c + 1) * 2048, 128:256], transpose=True)

            # ---- per query tile ----
            for qt in range(NQT):
                q0 = qt * 128
                qf = small.tile([128, D], F32, tag="qf")
                nc.sync.dma_start(qf[:], q_in[q0 : q0 + 128, :])
                qsq = small.tile([128, D], F32, tag="sqtmp")
                qn2 = small.tile([128, 1], F32, tag="qn2")
                nc.scalar.activation(qsq[:], qf[:], mybir.ActivationFunctionType.Square,
                                     accum_out=qn2[:])
                qnorm = small.tile([128, 1], F32, tag="qnorm")
                nc.scalar.activation(qnorm[:], qn2[:], mybir.ActivationFunctionType.Sqrt)
                nc.vector.tensor_scalar_max(qnorm[:], qnorm[:], 1e-12)
                qrin = small.tile([128, 1], F32, tag="qrin")
                nc.vector.reciprocal(qrin[:], qnorm[:])
                nqb = small.tile([128, D], BF16, tag="nqb")
                nc.scalar.activation(nqb[:], qf[:], mybir.ActivationFunctionType.Copy, scale=qrin[:])
                nc.sync.dma_start(qT_a[:], nqb[:, 0:128], transpose=True)
                nc.sync.dma_start(qT_b[:], nqb[:, 128:256], transpose=True)

                cand_v = small.tile([128, LCAND], F32, tag="cand_v")
                ci_u = small.tile([128, LCAND], U32, tag="ci_u")

                for s in range(NSEC):
                    sec = secp.tile([128, SEC], F32, tag="sec")
                    for ch in range(SEC // CH):
                        m0 = s * SEC + ch * CH
                        ps = psp.tile([128, CH], F32, tag="ps")
                        for b in range(CH // 512):
                            nc.tensor.matmul(
                                ps[:, b * 512 : (b + 1) * 512], qT_a[:],
                                nmT_a[:, m0 + b * 512 : m0 + (b + 1) * 512],
                                start=True, stop=False)
                        for b in range(CH // 512):
                            nc.tensor.matmul(
                                ps[:, b * 512 : (b + 1) * 512], qT_b[:],
                                nmT_b[:, m0 + b * 512 : m0 + (b + 1) * 512],
                                start=False, stop=True)
                        nc.scalar.copy(sec[:, ch * CH : (ch + 1) * CH], ps[:])
                    nc.vector.max(out=cand_v[:, s * 8 : (s + 1) * 8], in_=sec[:])
                    nc.vector.max_index(out=ci_u[:, s * 8 : (s + 1) * 8],
                                        in_max=cand_v[:, s * 8 : (s + 1) * 8],
                                        in_values=sec[:])

                cand_i = small.tile([128, LCAND], F32, tag="cand_i")
                nc.vector.tensor_copy(cand_i[:], ci_u[:])
                nc.vector.tensor_add(cand_i[:], cand_i[:], base_f[:])

                # self-match mask on screened values (fused is_ge*NEG)
                selfm = small.tile([128, LCAND], F32, tag="selfm")
                nc.vector.tensor_scalar(
                    out=selfm[:], in0=cand_v[:], scalar1=SELF_MATCH, scalar2=NEG,
                    op0=mybir.AluOpType.is_ge, op1=mybir.AluOpType.mult)
                nc.vector.tensor_add(cand_v[:], cand_v[:], selfm[:])

                pv1 = small.tile([128, 8], F32, tag="pv1")
                nc.vector.max(out=pv1[:], in_=cand_v[:])
                cand_v2 = small.tile([128, LCAND], F32, tag="cand_v2")
                nc.vector.match_replace(out=cand_v2[:], in_to_replace=pv1[:], in_values=cand_v[:], imm_value=NEG)
                pv2 = small.tile([128, 8], F32, tag="pv2")
                nc.vector.max(out=pv2[:], in_=cand_v2[:])

                pidx = small.tile([128, LP], F32, tag="pidx")
                mprod = small.tile([128, LCAND], F32, tag="mprod")
                for kk in range(LP):
                    pv = pv1 if kk < 8 else pv2
                    src_arr = cand_v if kk < 8 else cand_v2
                    nc.vector.scalar_tensor_tensor(
                        out=mprod[:], in0=src_arr[:],
                        scalar=pv[:, kk % 8 : kk % 8 + 1], in1=cand_i[:],
                        op0=mybir.AluOpType.is_equal, op1=mybir.AluOpType.mult,
                        accum_out=pidx[:, kk : kk + 1])

                pidx_u = small.tile([128, LP], U32, tag="pidx_u")
                nc.vector.tensor_copy(pidx_u[:], pidx[:])

                AW = D + 4
                G = gat.tile([128, LP * AW], F32, tag="G")
                GV = G[:].rearrange("p (c w) -> p c w", c=LP)
                for c in range(LP):
                    nc.gpsimd.indirect_dma_start(
                        out=G[:, c * AW : (c + 1) * AW],
                        out_offset=None,
                        in_=aug_dram[:],
                        in_offset=IndirectOffsetOnAxis(ap=pidx_u[:, c : c + 1], axis=0))

                nqf = small.tile([128, D], F32, tag="nqf")
                nc.scalar.activation(nqf[:], qf[:], mybir.ActivationFunctionType.Copy, scale=qrin[:])
                refined = small.tile([128, LP], F32, tag="refined")
                H = RH
                prod = res_pool.tile([128, H * D], F32, tag="prod")
                nqb3 = nqf[:].rearrange("p (c d) -> p c d", c=1).to_broadcast([128, H, D])
                for h in range(LP // RH):
                    g3 = GV[:, h * H:(h + 1) * H, 0:D]
                    nc.vector.tensor_tensor(
                        out=prod[:].rearrange("p (c d) -> p c d", c=H),
                        in0=g3, in1=nqb3, op=mybir.AluOpType.mult)
                    nc.vector.tensor_reduce(
                        out=refined[:, h * H : (h + 1) * H],
                        in_=prod[:].rearrange("p (c d) -> p c d", c=H),
                        axis=mybir.AxisListType.X, op=mybir.AluOpType.add)
                # refined /= ||m_c|| from the gathered norm column
                gn = small.tile([128, LP], F32, tag="gn")
                nc.vector.tensor_copy(
                    gn[:], GV[:, :, D:D + 1].rearrange("p c o -> p (c o)"))
                grin = small.tile([128, LP], F32, tag="grin")
                nc.vector.reciprocal(grin[:], gn[:])
                nc.vector.tensor_tensor(out=refined[:], in0=refined[:], in1=grin[:], op=mybir.AluOpType.mult)

                selfr = small.tile([128, LP], F32, tag="selfr")
                nc.vector.tensor_scalar(
                    out=selfr[:], in0=refined[:], scalar1=SELF_MATCH, scalar2=None,
                    op0=mybir.AluOpType.is_ge)
                nc.vector.tensor_scalar(
                    out=selfr[:], in0=selfr[:], scalar1=NEG, scalar2=None,
                    op0=mybir.AluOpType.mult)
                nc.vector.tensor_add(refined[:], refined[:], selfr[:])

                top8 = small.tile([128, 8], F32, tag="top8")
                nc.vector.max(out=top8[:], in_=refined[:])
                wmask = small.tile([128, LP], F32, tag="wmask")
                nc.vector.tensor_scalar(
                    out=wmask[:], in0=refined[:], scalar1=top8[:, 7:8],
                    scalar2=None, op0=mybir.AluOpType.is_ge)
                shift = small.tile([128, LP], F32, tag="shift")
                nc.vector.tensor_scalar(
                    out=shift[:], in0=refined[:], scalar1=top8[:, 0:1],
                    scalar2=None, op0=mybir.AluOpType.subtract)
                expv = small.tile([128, LP], F32, tag="expv")
                nc.scalar.activation(expv[:], shift[:], mybir.ActivationFunctionType.Exp)
                wts = small.tile([128, LP], F32, tag="wts")
                nc.vector.tensor_tensor(out=wts[:], in0=expv[:], in1=wmask[:], op=mybir.AluOpType.mult)
                zsum = small.tile([128, 1], F32, tag="zsum")
                nc.vector.tensor_reduce(out=zsum[:], in_=wts[:], axis=mybir.AxisListType.X, op=mybir.AluOpType.add)
                zrin = small.tile([128, 1], F32, tag="zrin")
                nc.vector.reciprocal(zrin[:], zsum[:])
                nc.vector.tensor_scalar(
                    out=wts[:], in0=wts[:], scalar1=zrin[:, 0:1], scalar2=None,
                    op0=mybir.AluOpType.mult)

                nc.vector.scalar_tensor_tensor(
                    out=GV[:, :, 0:D],
                    in0=GV[:, :, 0:D], scalar=1.0,
                    in1=wts[:].rearrange("p (c o) -> p c o", o=1)
                    .to_broadcast([128, LP, D]),
                    op0=mybir.AluOpType.mult, op1=mybir.AluOpType.mult)
                w = LP
                while w > 1:
                    nw = w // 2
                    nc.vector.tensor_add(GV[:, 0:nw, 0:D], GV[:, 0:nw, 0:D],
                                         GV[:, nw:w, 0:D])
                    w = nw
                acc = small.tile([128, D], F32, tag="acc")
                nc.vector.tensor_copy(acc[:], G[:, 0:D])

                asq = small.tile([128, D], F32, tag="sqtmp")
                an2 = small.tile([128, 1], F32, tag="an2")
                nc.scalar.activation(asq[:], acc[:], mybir.ActivationFunctionType.Square,
                                     accum_out=an2[:])
                an = small.tile([128, 1], F32, tag="an")
                nc.scalar.activation(an[:], an2[:], mybir.ActivationFunctionType.Sqrt)
                nc.vector.tensor_scalar_max(an[:], an[:], 1e-12)
                arin = small.tile([128, 1], F32, tag="arin")
                nc.vector.reciprocal(arin[:], an[:])
                scl = small.tile([128, 1], F32, tag="scl")
                nc.vector.tensor_tensor(out=scl[:], in0=arin[:], in1=qnorm[:], op=mybir.AluOpType.mult)
                ot = small.tile([128, D], F32, tag="ot")
                nc.scalar.activation(ot[:], acc[:], mybir.ActivationFunctionType.Copy, scale=scl[:])
                nc.sync.dma_start(out[q0 : q0 + 128, :], ot[:])

    return nc


_CACHED_NC = None


def _get_nc():
    global _CACHED_NC
    if _CACHED_NC is None:
        _install_patches()
        _CACHED_NC = _build()
    return _CACHED_NC


def kernel(query, memory, k):
    query = np.ascontiguousarray(np.asarray(query, dtype=np.float32))
    memory = np.ascontiguousarray(np.asarray(memory, dtype=np.float32))
    k_val = int(np.asarray(k))
    assert query.shape == (B, D) and memory.shape == (M, D), (query.shape, memory.shape)
    assert k_val == K, f"kernel compiled for k={K}, got {k_val}"

    from concourse.bass_utils import run_bass_kernel_spmd

    nc = _get_nc()
    in_maps = [
        {"q": query[i * B_LOC : (i + 1) * B_LOC], "mem": memory}
        for i in range(N_CORES)
    ]
    res = run_bass_kernel_spmd(nc, in_maps, list(range(N_CORES)))
    return np.concatenate([res.results[i]["out"] for i in range(N_CORES)], axis=0)



# revision 11
# speedup vs baseline: 1.0454x; 1.0454x over previous
"""Trainium2 Bass kernel for nn_MemoryBank_51135880626820 (scatter_memory).

Data-parallel over the query batch across 8 NeuronCores: the [32768, 256]
memory bank is replicated per core, each core handles 1024 query rows.

Per-core pipeline:
  - normalize memory rows, cast bf16, DMA-transpose roundtrip to build a
    resident d-major [256, 32768] bf16 bank (two [128, 32768] SBUF tiles)
  - per 128-query tile: bf16 matmul (fp32 PSUM) -> ACT evacuation to fp32
    SBUF sectors -> DVE max8/max_index screening -> prune to 16 candidates
    -> indirect-DMA gather of candidate memory rows (fp32) -> exact fp32
    re-dot on DVE -> top-8 + softmax + weighted sum -> renormalize to ||q||.

self-contained: hardcodes all shapes; builds and caches the Bass program on
first call.
"""

import sys

for _p in ("/opt/trn_rl_repo",):
    if _p not in sys.path:
        sys.path.insert(0, _p)

import numpy as np

import concourse.bass as bass
import concourse.mybir as mybir
import concourse.tile as tile
from concourse.bass import IndirectOffsetOnAxis

F32 = mybir.dt.float32
BF16 = mybir.dt.bfloat16
U32 = mybir.dt.uint32

N_CORES = 8
B = 8192
B_LOC = B // N_CORES        # 1024
M = 32768
D = 256
K = 8
NQT = B_LOC // 128          # 8 query tiles per core
SEC = 2048                  # screening sector width (f32 SBUF)
NSEC = M // SEC
CH = 2048                   # PSUM evacuation chunk (4 banks)
LCAND = NSEC * 8            # 64 raw candidates
LP = 16                     # pruned candidates (refined exactly)
RH = 8                      # refine sub-batch (SBUF-fit)
NEG = -1.0e30
SELF_MATCH = 0.9999


# --------------------------------------------------------------------------
# workarounds for this container's walrus build, which rejects more than one
# sync-wait per instruction ("Too many sync wait commands").
# --------------------------------------------------------------------------
def _install_patches():
    import json

    import bass_rust
    import concourse.bass_utils as _bu
    import concourse.bass2jax as _b2j
    import concourse.tile as tile_mod
    from concourse.tile import TileContext

    if getattr(_bu, "_mb_patched", False):
        return

    try:
        ScopedClock = tile_mod.ScopedClock
    except AttributeError:
        ScopedClock = bass_rust.ScopedClock

    def _patched_drain_and_barrier(self, tick_clock, wait_clock):
        nc = self.nc
        drain_inst = nc.sync.drain()
        wait_clock.add_sem_waits(
            drain_inst.ins, ScopedClock({None: tick_clock.global_clock})
        )
        si = drain_inst.ins.sync_info
        waits = list(si.on_wait) if si is not None and si.on_wait else []
        if len(waits) > 1:
            drain_inst.ins.sync_info = bass_rust.SyncInfo(
                on_wait=[waits[0]],
                on_update=list(si.on_update) if si.on_update else [],
            )
            for w in waits[1:]:
                nop = nc.sync.nop(nofuse=True, hint="tail_wait")
                nop.ins.sync_info = bass_rust.SyncInfo(on_wait=[w], on_update=[])
        nc.all_engine_barrier()
        assert self.sems is not None
        popped = nc._tile_sem_poison_stack.pop()
        assert popped is self._sem_poison
        nc.clear_and_free_semaphores(list(self.sems.allocated().values()))
        nc.all_engine_barrier()

    TileContext._drain_and_barrier = _patched_drain_and_barrier

    def split_multiwaits(bir_json):
        m = json.loads(bir_json)
        changed = False
        for fn in m.get("functions", []):
            for bb in fn.get("blocks", []):
                insts = bb.get("instructions", [])
                out = []
                for ins in insts:
                    si = ins.get("sync_info") or {}
                    waits = si.get("on_wait") or []
                    if len(waits) > 1:
                        changed = True
                        for kk, w in enumerate(waits[:-1]):
                            out.append({
                                "debug": ins.get("debug", 0),
                                "engine": ins["engine"],
                                "ins": [],
                                "name": f"{ins['name']}-w{kk}",
                                "opcode": "NoOp",
                                "outs": [],
                                "sync_info": {"on_update": [], "on_wait": [w]},
                                "text_hint": "split_wait",
                            })
                        si = dict(si)
                        si["on_wait"] = [waits[-1]]
                        ins = dict(ins)
                        ins["sync_info"] = si
                    out.append(ins)
                bb["instructions"] = out
        if not changed:
            return bir_json
        return json.dumps(m).encode()

    _orig_compile = _bu.compile_bir_kernel

    def _patched_compile(bir_json, tmpdir, neff_name="file.neff"):
        if isinstance(bir_json, str):
            bir_json = bir_json.encode()
        return _orig_compile(split_multiwaits(bir_json), tmpdir, neff_name)

    _bu.compile_bir_kernel = _patched_compile
    _b2j.compile_bir_kernel = _patched_compile
    _bu._mb_patched = True


# --------------------------------------------------------------------------
# per-core Bass program
# --------------------------------------------------------------------------
def _build():
    nc = bass.Bass("TRN2", target_bir_lowering=False, debug=False)
    q_in = nc.dram_tensor("q", [B_LOC, D], F32, kind="ExternalInput")
    mem_in = nc.dram_tensor("mem", [M, D], F32, kind="ExternalInput")
    out = nc.dram_tensor("out", [B_LOC, D], F32, kind="ExternalOutput")
    nm_dram = nc.dram_tensor("nm_dram", [M, D], BF16)

    with tile.TileContext(nc) as tc:
        with (
            tc.tile_pool(name="resident", bufs=1) as res_pool,
            tc.tile_pool(name="mprep", bufs=2) as mprep,
            tc.tile_pool(name="sector", bufs=2) as secp,
            tc.tile_pool(name="small", bufs=2) as small,
            tc.tile_pool(name="gat", bufs=1) as gat,
            tc.tile_pool(name="psum", bufs=2, space="PSUM") as psp,
        ):
            nmT_a = res_pool.tile([128, M], BF16, tag="nmT_a")
            nmT_b = res_pool.tile([128, M], BF16, tag="nmT_b")
            qT_a = res_pool.tile([128, 128], BF16, tag="qT_a")
            qT_b = res_pool.tile([128, 128], BF16, tag="qT_b")

            # sector base indices: value s*SEC repeated 8, as f32
            base_u = res_pool.tile([128, LCAND], mybir.dt.uint32, tag="base_u")
            nc.gpsimd.iota(base_u[:], pattern=[[SEC, NSEC], [0, 8]], base=0,
                           channel_multiplier=0)
            base_f = res_pool.tile([128, LCAND], F32, tag="base_f")
            nc.vector.tensor_copy(base_f[:], base_u[:])

            # ---- memory prep: grouped 1024-row iterations ----
            GR = 4  # row-tiles per group
            for g in range(M // (128 * GR)):
                r0 = g * 128 * GR
                mtg = mprep.tile([128, GR * D], F32, tag="mtg")
                nc.sync.dma_start(
                    mtg[:].rearrange("p (t d) -> p t d", t=GR),
                    mem_in[r0:r0 + 128 * GR, :].rearrange("(t p) d -> p t d", p=128))
                sq = mprep.tile([128, D], F32, tag="sq")
                nrm2 = mprep.tile([128, GR], F32, tag="nrm2")
                for tl in range(GR):
                    nc.scalar.activation(sq[:], mtg[:, tl * D:(tl + 1) * D],
                                         mybir.ActivationFunctionType.Square,
                                         accum_out=nrm2[:, tl:tl + 1])
                nrm = mprep.tile([128, GR], F32, tag="nrm")
                nc.scalar.activation(nrm[:], nrm2[:], mybir.ActivationFunctionType.Sqrt)
                nc.vector.tensor_scalar_max(nrm[:], nrm[:], 1e-12)
                rin = mprep.tile([128, GR], F32, tag="rin")
                nc.vector.reciprocal(rin[:], nrm[:])
                nmbg = mprep.tile([128, GR * D], BF16, tag="nmbg")
                for tl in range(GR):
                    nc.vector.tensor_scalar(
                        out=nmbg[:, tl * D:(tl + 1) * D],
                        in0=mtg[:, tl * D:(tl + 1) * D],
                        scalar1=rin[:, tl:tl + 1], scalar2=None,
                        op0=mybir.AluOpType.mult)
                nc.sync.dma_start(
                    nm_dram[r0:r0 + 128 * GR, :].rearrange("(t p) d -> p t d", p=128),
                    nmbg[:].rearrange("p (t d) -> p t d", t=GR))

            for c in range(M // 2048):
                nc.sync.dma_start(
                    nmT_a[:, c * 2048 : (c + 1) * 2048],
                    nm_dram[c * 2048 : (c + 1) * 2048, 0:128], transpose=True)
                nc.sync.dma_start(
                    nmT_b[:, c * 2048 : (c + 1) * 2048],
                    nm_dram[c * 2048 : (c + 1) * 2048, 128:256], transpose=True)

            # ---- per query tile ----
            for qt in range(NQT):
                q0 = qt * 128
                qf = small.tile([128, D], F32, tag="qf")
                nc.sync.dma_start(qf[:], q_in[q0 : q0 + 128, :])
                qsq = small.tile([128, D], F32, tag="sqtmp")
                qn2 = small.tile([128, 1], F32, tag="qn2")
                nc.scalar.activation(qsq[:], qf[:], mybir.ActivationFunctionType.Square,
                                     accum_out=qn2[:])
                qnorm = small.tile([128, 1], F32, tag="qnorm")
                nc.scalar.activation(qnorm[:], qn2[:], mybir.ActivationFunctionType.Sqrt)
                nc.vector.tensor_scalar_max(qnorm[:], qnorm[:], 1e-12)
                qrin = small.tile([128, 1], F32, tag="qrin")
                nc.vector.reciprocal(qrin[:], qnorm[:])
                nqb = small.tile([128, D], BF16, tag="nqb")
                nc.scalar.activation(nqb[:], qf[:], mybir.ActivationFunctionType.Copy, scale=qrin[:])
                nc.sync.dma_start(qT_a[:], nqb[:, 0:128], transpose=True)
                nc.sync.dma_start(qT_b[:], nqb[:, 128:256], transpose=True)

                cand_v = small.tile([128, LCAND], F32, tag="cand_v")
                ci_u = small.tile([128, LCAND], U32, tag="ci_u")

                for s in range(NSEC):
                    sec = secp.tile([128, SEC], F32, tag="sec")
                    for ch in range(SEC // CH):
                        m0 = s * SEC + ch * CH
                        ps = psp.tile([128, CH], F32, tag="ps")
                        for b in range(CH // 512):
                            nc.tensor.matmul(
                                ps[:, b * 512 : (b + 1) * 512], qT_a[:],
                                nmT_a[:, m0 + b * 512 : m0 + (b + 1) * 512],
                                start=True, stop=False)
                        for b in range(CH // 512):
                            nc.tensor.matmul(
                                ps[:, b * 512 : (b + 1) * 512], qT_b[:],
                                nmT_b[:, m0 + b * 512 : m0 + (b + 1) * 512],
                                start=False, stop=True)
                        nc.scalar.copy(sec[:, ch * CH : (ch + 1) * CH], ps[:])
                    nc.vector.max(out=cand_v[:, s * 8 : (s + 1) * 8], in_=sec[:])
                    nc.vector.max_index(out=ci_u[:, s * 8 : (s + 1) * 8],
                                        in_max=cand_v[:, s * 8 : (s + 1) * 8],
                                        in_values=sec[:])

                cand_i = small.tile([128, LCAND], F32, tag="cand_i")
                nc.vector.tensor_copy(cand_i[:], ci_u[:])
                nc.vector.tensor_add(cand_i[:], cand_i[:], base_f[:])

                # self-match mask on screened values (fused is_ge*NEG)
                selfm = small.tile([128, LCAND], F32, tag="selfm")
                nc.vector.tensor_scalar(
                    out=selfm[:], in0=cand_v[:], scalar1=SELF_MATCH, scalar2=NEG,
                    op0=mybir.AluOpType.is_ge, op1=mybir.AluOpType.mult)
                nc.vector.tensor_add(cand_v[:], cand_v[:], selfm[:])

                pv1 = small.tile([128, 8], F32, tag="pv1")
                nc.vector.max(out=pv1[:], in_=cand_v[:])
                cand_v2 = small.tile([128, LCAND], F32, tag="cand_v2")
                nc.vector.match_replace(out=cand_v2[:], in_to_replace=pv1[:], in_values=cand_v[:], imm_value=NEG)
                pv2 = small.tile([128, 8], F32, tag="pv2")
                nc.vector.max(out=pv2[:], in_=cand_v2[:])

                pidx = small.tile([128, LP], F32, tag="pidx")
                mprod = small.tile([128, LCAND], F32, tag="mprod")
                for kk in range(LP):
                    pv = pv1 if kk < 8 else pv2
                    src_arr = cand_v if kk < 8 else cand_v2
                    nc.vector.scalar_tensor_tensor(
                        out=mprod[:], in0=src_arr[:],
                        scalar=pv[:, kk % 8 : kk % 8 + 1], in1=cand_i[:],
                        op0=mybir.AluOpType.is_equal, op1=mybir.AluOpType.mult,
                        accum_out=pidx[:, kk : kk + 1])

                pidx_u = small.tile([128, LP], U32, tag="pidx_u")
                nc.vector.tensor_copy(pidx_u[:], pidx[:])

                G = gat.tile([128, LP * D], F32, tag="G")
                GV = G[:].rearrange("p (c w) -> p c w", c=LP)
                for c in range(LP):
                    nc.gpsimd.indirect_dma_start(
                        out=G[:, c * D : (c + 1) * D],
                        out_offset=None,
                        in_=mem_in[:],
                        in_offset=IndirectOffsetOnAxis(ap=pidx_u[:, c : c + 1], axis=0))

                nqf = small.tile([128, D], F32, tag="nqf")
                nc.scalar.activation(nqf[:], qf[:], mybir.ActivationFunctionType.Copy, scale=qrin[:])
                refined = small.tile([128, LP], F32, tag="refined")
                H = RH
                prod = res_pool.tile([128, H * D], F32, tag="prod")
                nqb3 = nqf[:].rearrange("p (c d) -> p c d", c=1).to_broadcast([128, H, D])
                for h in range(LP // RH):
                    g3 = GV[:, h * H:(h + 1) * H, 0:D]
                    nc.vector.tensor_tensor(
                        out=prod[:].rearrange("p (c d) -> p c d", c=H),
                        in0=g3, in1=nqb3, op=mybir.AluOpType.mult)
                    nc.vector.tensor_reduce(
                        out=refined[:, h * H : (h + 1) * H],
                        in_=prod[:].rearrange("p (c d) -> p c d", c=H),
                        axis=mybir.AxisListType.X, op=mybir.AluOpType.add)
                # candidate norms via G*G (second pass through prod)
                gn2 = small.tile([128, LP], F32, tag="gn2")
                for h in range(LP // RH):
                    g3 = GV[:, h * RH:(h + 1) * RH, 0:D]
                    nc.vector.tensor_tensor(
                        out=prod[:].rearrange("p (c d) -> p c d", c=RH),
                        in0=g3, in1=g3, op=mybir.AluOpType.mult)
                    nc.vector.tensor_reduce(
                        out=gn2[:, h * RH : (h + 1) * RH],
                        in_=prod[:].rearrange("p (c d) -> p c d", c=RH),
                        axis=mybir.AxisListType.X, op=mybir.AluOpType.add)
                gn = small.tile([128, LP], F32, tag="gn")
                nc.scalar.activation(gn[:], gn2[:], mybir.ActivationFunctionType.Sqrt)
                nc.vector.tensor_scalar_max(gn[:], gn[:], 1e-12)
                grin = small.tile([128, LP], F32, tag="grin")
                nc.vector.reciprocal(grin[:], gn[:])
                nc.vector.tensor_tensor(out=refined[:], in0=refined[:], in1=grin[:], op=mybir.AluOpType.mult)

                selfr = small.tile([128, LP], F32, tag="selfr")
                nc.vector.tensor_scalar(
                    out=selfr[:], in0=refined[:], scalar1=SELF_MATCH, scalar2=None,
                    op0=mybir.AluOpType.is_ge)
                nc.vector.tensor_scalar(
                    out=selfr[:], in0=selfr[:], scalar1=NEG, scalar2=None,
                    op0=mybir.AluOpType.mult)
                nc.vector.tensor_add(refined[:], refined[:], selfr[:])

                top8 = small.tile([128, 8], F32, tag="top8")
                nc.vector.max(out=top8[:], in_=refined[:])
                wmask = small.tile([128, LP], F32, tag="wmask")
                nc.vector.tensor_scalar(
                    out=wmask[:], in0=refined[:], scalar1=top8[:, 7:8],
                    scalar2=None, op0=mybir.AluOpType.is_ge)
                shift = small.tile([128, LP], F32, tag="shift")
                nc.vector.tensor_scalar(
                    out=shift[:], in0=refined[:], scalar1=top8[:, 0:1],
                    scalar2=None, op0=mybir.AluOpType.subtract)
                expv = small.tile([128, LP], F32, tag="expv")
                nc.scalar.activation(expv[:], shift[:], mybir.ActivationFunctionType.Exp)
                wts = small.tile([128, LP], F32, tag="wts")
                nc.vector.tensor_tensor(out=wts[:], in0=expv[:], in1=wmask[:], op=mybir.AluOpType.mult)
                zsum = small.tile([128, 1], F32, tag="zsum")
                nc.vector.tensor_reduce(out=zsum[:], in_=wts[:], axis=mybir.AxisListType.X, op=mybir.AluOpType.add)
                zrin = small.tile([128, 1], F32, tag="zrin")
                nc.vector.reciprocal(zrin[:], zsum[:])
                nc.vector.tensor_scalar(
                    out=wts[:], in0=wts[:], scalar1=zrin[:, 0:1], scalar2=None,
                    op0=mybir.AluOpType.mult)

                nc.vector.scalar_tensor_tensor(
                    out=GV[:, :, 0:D],
                    in0=GV[:, :, 0:D], scalar=1.0,
                    in1=wts[:].rearrange("p (c o) -> p c o", o=1)
                    .to_broadcast([128, LP, D]),
                    op0=mybir.AluOpType.mult, op1=mybir.AluOpType.mult)
                w = LP
                while w > 1:
                    nw = w // 2
                    nc.vector.tensor_add(GV[:, 0:nw, 0:D], GV[:, 0:nw, 0:D],
                                         GV[:, nw:w, 0:D])
                    w = nw
                acc = small.tile([128, D], F32, tag="acc")
                nc.vector.tensor_copy(acc[:], G[:, 0:D])

                asq = small.tile([128, D], F32, tag="sqtmp")
                an2 = small.tile([128, 1], F32, tag="an2")
                nc.scalar.activation(asq[:], acc[:], mybir.ActivationFunctionType.Square,
                                     accum_out=an2[:])
                an = small.tile([128, 1], F32, tag="an")
                nc.scalar.activation(an[:], an2[:], mybir.ActivationFunctionType.Sqrt)
                nc.vector.tensor_scalar_max(an[:], an[:], 1e-12)
                arin = small.tile([128, 1], F32, tag="arin")
                nc.vector.reciprocal(arin[:], an[:])
                scl = small.tile([128, 1], F32, tag="scl")
                nc.vector.tensor_tensor(out=scl[:], in0=arin[:], in1=qnorm[:], op=mybir.AluOpType.mult)
                ot = small.tile([128, D], F32, tag="ot")
                nc.scalar.activation(ot[:], acc[:], mybir.ActivationFunctionType.Copy, scale=scl[:])
                nc.sync.dma_start(out[q0 : q0 + 128, :], ot[:])

    return nc


_CACHED_NC = None


def _get_nc():
    global _CACHED_NC
    if _CACHED_NC is None:
        _install_patches()
        _CACHED_NC = _build()
    return _CACHED_NC


def kernel(query, memory, k):
    query = np.ascontiguousarray(np.asarray(query, dtype=np.float32))
    memory = np.ascontiguousarray(np.asarray(memory, dtype=np.float32))
    k_val = int(np.asarray(k))
    assert query.shape == (B, D) and memory.shape == (M, D), (query.shape, memory.shape)
    assert k_val == K, f"kernel compiled for k={K}, got {k_val}"

    from concourse.bass_utils import run_bass_kernel_spmd

    nc = _get_nc()
    in_maps = [
        {"q": query[i * B_LOC : (i + 1) * B_LOC], "mem": memory}
        for i in range(N_CORES)
    ]
    res = run_bass_kernel_spmd(nc, in_maps, list(range(N_CORES)))
    return np.concatenate([res.results[i]["out"] for i in range(N_CORES)], axis=0)



# revision 12
# speedup vs baseline: 1.0781x; 1.0313x over previous
"""Trainium2 Bass kernel for nn_MemoryBank_51135880626820 (scatter_memory).

Data-parallel over the query batch across 8 NeuronCores: the [32768, 256]
memory bank is replicated per core, each core handles 1024 query rows.

Per-core pipeline:
  - normalize memory rows, cast bf16, DMA-transpose roundtrip to build a
    resident d-major [256, 32768] bf16 bank (two [128, 32768] SBUF tiles)
  - per 128-query tile: bf16 matmul (fp32 PSUM) -> ACT evacuation to fp32
    SBUF sectors -> DVE max8/max_index screening -> prune to 16 candidates
    -> indirect-DMA gather of candidate memory rows (fp32) -> exact fp32
    re-dot on DVE -> top-8 + softmax + weighted sum -> renormalize to ||q||.

self-contained: hardcodes all shapes; builds and caches the Bass program on
first call.
"""

import sys

for _p in ("/opt/trn_rl_repo",):
    if _p not in sys.path:
        sys.path.insert(0, _p)

import numpy as np

import concourse.bass as bass
import concourse.mybir as mybir
import concourse.tile as tile
from concourse.bass import IndirectOffsetOnAxis

F32 = mybir.dt.float32
BF16 = mybir.dt.bfloat16
U32 = mybir.dt.uint32

N_CORES = 8
B = 8192
B_LOC = B // N_CORES        # 1024
M = 32768
D = 256
K = 8
NQT = B_LOC // 128          # 8 query tiles per core
SEC = 2048                  # screening sector width (f32 SBUF)
NSEC = M // SEC
CH = 2048                   # PSUM evacuation chunk (4 banks)
LCAND = NSEC * 8            # 64 raw candidates
LP = 16                     # pruned candidates (refined exactly)
RH = 8                      # refine sub-batch (SBUF-fit)
NEG = -1.0e30
SELF_MATCH = 0.9999


# --------------------------------------------------------------------------
# workarounds for this container's walrus build, which rejects more than one
# sync-wait per instruction ("Too many sync wait commands").
# --------------------------------------------------------------------------
def _install_patches():
    import json

    import bass_rust
    import concourse.bass_utils as _bu
    import concourse.bass2jax as _b2j
    import concourse.tile as tile_mod
    from concourse.tile import TileContext

    if getattr(_bu, "_mb_patched", False):
        return

    try:
        ScopedClock = tile_mod.ScopedClock
    except AttributeError:
        ScopedClock = bass_rust.ScopedClock

    def _patched_drain_and_barrier(self, tick_clock, wait_clock):
        nc = self.nc
        drain_inst = nc.sync.drain()
        wait_clock.add_sem_waits(
            drain_inst.ins, ScopedClock({None: tick_clock.global_clock})
        )
        si = drain_inst.ins.sync_info
        waits = list(si.on_wait) if si is not None and si.on_wait else []
        if len(waits) > 1:
            drain_inst.ins.sync_info = bass_rust.SyncInfo(
                on_wait=[waits[0]],
                on_update=list(si.on_update) if si.on_update else [],
            )
            for w in waits[1:]:
                nop = nc.sync.nop(nofuse=True, hint="tail_wait")
                nop.ins.sync_info = bass_rust.SyncInfo(on_wait=[w], on_update=[])
        nc.all_engine_barrier()
        assert self.sems is not None
        popped = nc._tile_sem_poison_stack.pop()
        assert popped is self._sem_poison
        nc.clear_and_free_semaphores(list(self.sems.allocated().values()))
        nc.all_engine_barrier()

    TileContext._drain_and_barrier = _patched_drain_and_barrier

    def split_multiwaits(bir_json):
        m = json.loads(bir_json)
        changed = False
        for fn in m.get("functions", []):
            for bb in fn.get("blocks", []):
                insts = bb.get("instructions", [])
                out = []
                for ins in insts:
                    si = ins.get("sync_info") or {}
                    waits = si.get("on_wait") or []
                    if len(waits) > 1:
                        changed = True
                        for kk, w in enumerate(waits[:-1]):
                            out.append({
                                "debug": ins.get("debug", 0),
                                "engine": ins["engine"],
                                "ins": [],
                                "name": f"{ins['name']}-w{kk}",
                                "opcode": "NoOp",
                                "outs": [],
                                "sync_info": {"on_update": [], "on_wait": [w]},
                                "text_hint": "split_wait",
                            })
                        si = dict(si)
                        si["on_wait"] = [waits[-1]]
                        ins = dict(ins)
                        ins["sync_info"] = si
                    out.append(ins)
                bb["instructions"] = out
        if not changed:
            return bir_json
        return json.dumps(m).encode()

    _orig_compile = _bu.compile_bir_kernel

    def _patched_compile(bir_json, tmpdir, neff_name="file.neff"):
        if isinstance(bir_json, str):
            bir_json = bir_json.encode()
        return _orig_compile(split_multiwaits(bir_json), tmpdir, neff_name)

    _bu.compile_bir_kernel = _patched_compile
    _b2j.compile_bir_kernel = _patched_compile
    _bu._mb_patched = True


# --------------------------------------------------------------------------
# per-core Bass program
# --------------------------------------------------------------------------
def _build():
    nc = bass.Bass("TRN2", target_bir_lowering=False, debug=False)
    q_in = nc.dram_tensor("q", [B_LOC, D], F32, kind="ExternalInput")
    mem_in = nc.dram_tensor("mem", [M, D], F32, kind="ExternalInput")
    out = nc.dram_tensor("out", [B_LOC, D], F32, kind="ExternalOutput")
    nm_dram = nc.dram_tensor("nm_dram", [M, D], BF16)

    with tile.TileContext(nc) as tc:
        with (
            tc.tile_pool(name="resident", bufs=1) as res_pool,
            tc.tile_pool(name="mprep", bufs=2) as mprep,
            tc.tile_pool(name="sector", bufs=2) as secp,
            tc.tile_pool(name="small", bufs=2) as small,
            tc.tile_pool(name="gat", bufs=1) as gat,
            tc.tile_pool(name="psum", bufs=2, space="PSUM") as psp,
        ):
            nmT_a = res_pool.tile([128, M], BF16, tag="nmT_a")
            nmT_b = res_pool.tile([128, M], BF16, tag="nmT_b")
            qT_a = res_pool.tile([128, 128], BF16, tag="qT_a")
            qT_b = res_pool.tile([128, 128], BF16, tag="qT_b")

            # sector base indices: value s*SEC repeated 8, as f32
            base_u = res_pool.tile([128, LCAND], mybir.dt.uint32, tag="base_u")
            nc.gpsimd.iota(base_u[:], pattern=[[SEC, NSEC], [0, 8]], base=0,
                           channel_multiplier=0)
            base_f = res_pool.tile([128, LCAND], F32, tag="base_f")
            nc.vector.tensor_copy(base_f[:], base_u[:])

            # ---- memory prep: grouped 1024-row iterations ----
            GR = 4  # row-tiles per group
            for g in range(M // (128 * GR)):
                r0 = g * 128 * GR
                mtg = mprep.tile([128, GR * D], F32, tag="mtg")
                nc.sync.dma_start(
                    mtg[:].rearrange("p (t d) -> p t d", t=GR),
                    mem_in[r0:r0 + 128 * GR, :].rearrange("(t p) d -> p t d", p=128))
                sq = mprep.tile([128, D], F32, tag="sq")
                nrm2 = mprep.tile([128, GR], F32, tag="nrm2")
                for tl in range(GR):
                    nc.scalar.activation(sq[:], mtg[:, tl * D:(tl + 1) * D],
                                         mybir.ActivationFunctionType.Square,
                                         accum_out=nrm2[:, tl:tl + 1])
                nrm = mprep.tile([128, GR], F32, tag="nrm")
                nc.scalar.activation(nrm[:], nrm2[:], mybir.ActivationFunctionType.Sqrt)
                nc.vector.tensor_scalar_max(nrm[:], nrm[:], 1e-12)
                rin = mprep.tile([128, GR], F32, tag="rin")
                nc.vector.reciprocal(rin[:], nrm[:])
                nmbg = mprep.tile([128, GR * D], BF16, tag="nmbg")
                for tl in range(GR):
                    nc.vector.tensor_scalar(
                        out=nmbg[:, tl * D:(tl + 1) * D],
                        in0=mtg[:, tl * D:(tl + 1) * D],
                        scalar1=rin[:, tl:tl + 1], scalar2=None,
                        op0=mybir.AluOpType.mult)
                nc.sync.dma_start(
                    nm_dram[r0:r0 + 128 * GR, :].rearrange("(t p) d -> p t d", p=128),
                    nmbg[:].rearrange("p (t d) -> p t d", t=GR))

            for c in range(M // 2048):
                nc.sync.dma_start(
                    nmT_a[:, c * 2048 : (c + 1) * 2048],
                    nm_dram[c * 2048 : (c + 1) * 2048, 0:128], transpose=True)
                nc.sync.dma_start(
                    nmT_b[:, c * 2048 : (c + 1) * 2048],
                    nm_dram[c * 2048 : (c + 1) * 2048, 128:256], transpose=True)

            # ---- per query tile ----
            for qt in range(NQT):
                q0 = qt * 128
                qf = small.tile([128, D], F32, tag="qf")
                nc.sync.dma_start(qf[:], q_in[q0 : q0 + 128, :])
                qsq = small.tile([128, D], F32, tag="sqtmp")
                qn2 = small.tile([128, 1], F32, tag="qn2")
                nc.scalar.activation(qsq[:], qf[:], mybir.ActivationFunctionType.Square,
                                     accum_out=qn2[:])
                qnorm = small.tile([128, 1], F32, tag="qnorm")
                nc.scalar.activation(qnorm[:], qn2[:], mybir.ActivationFunctionType.Sqrt)
                nc.vector.tensor_scalar_max(qnorm[:], qnorm[:], 1e-12)
                qrin = small.tile([128, 1], F32, tag="qrin")
                nc.vector.reciprocal(qrin[:], qnorm[:])
                nqb = small.tile([128, D], BF16, tag="nqb")
                nc.scalar.activation(nqb[:], qf[:], mybir.ActivationFunctionType.Copy, scale=qrin[:])
                nc.sync.dma_start(qT_a[:], nqb[:, 0:128], transpose=True)
                nc.sync.dma_start(qT_b[:], nqb[:, 128:256], transpose=True)

                cand_v = small.tile([128, LCAND], F32, tag="cand_v")
                ci_u = small.tile([128, LCAND], U32, tag="ci_u")

                for s in range(NSEC):
                    sec = secp.tile([128, SEC], F32, tag="sec")
                    for ch in range(SEC // CH):
                        m0 = s * SEC + ch * CH
                        ps = psp.tile([128, CH], F32, tag="ps")
                        for b in range(CH // 512):
                            nc.tensor.matmul(
                                ps[:, b * 512 : (b + 1) * 512], qT_a[:],
                                nmT_a[:, m0 + b * 512 : m0 + (b + 1) * 512],
                                start=True, stop=False)
                        for b in range(CH // 512):
                            nc.tensor.matmul(
                                ps[:, b * 512 : (b + 1) * 512], qT_b[:],
                                nmT_b[:, m0 + b * 512 : m0 + (b + 1) * 512],
                                start=False, stop=True)
                        nc.scalar.copy(sec[:, ch * CH : (ch + 1) * CH], ps[:])
                    nc.vector.max(out=cand_v[:, s * 8 : (s + 1) * 8], in_=sec[:])
                    nc.vector.max_index(out=ci_u[:, s * 8 : (s + 1) * 8],
                                        in_max=cand_v[:, s * 8 : (s + 1) * 8],
                                        in_values=sec[:])

                cand_i = small.tile([128, LCAND], F32, tag="cand_i")
                nc.vector.tensor_copy(cand_i[:], ci_u[:])
                nc.vector.tensor_add(cand_i[:], cand_i[:], base_f[:])

                # self-match mask on screened values (fused is_ge*NEG)
                selfm = small.tile([128, LCAND], F32, tag="selfm")
                nc.vector.tensor_scalar(
                    out=selfm[:], in0=cand_v[:], scalar1=SELF_MATCH, scalar2=NEG,
                    op0=mybir.AluOpType.is_ge, op1=mybir.AluOpType.mult)
                nc.vector.tensor_add(cand_v[:], cand_v[:], selfm[:])

                pv1 = small.tile([128, 8], F32, tag="pv1")
                nc.vector.max(out=pv1[:], in_=cand_v[:])
                cand_v2 = small.tile([128, LCAND], F32, tag="cand_v2")
                nc.vector.match_replace(out=cand_v2[:], in_to_replace=pv1[:], in_values=cand_v[:], imm_value=NEG)
                pv2 = small.tile([128, 8], F32, tag="pv2")
                nc.vector.max(out=pv2[:], in_=cand_v2[:])

                pidx = small.tile([128, LP], F32, tag="pidx")
                mprod = small.tile([128, LCAND], F32, tag="mprod")
                for kk in range(LP):
                    pv = pv1 if kk < 8 else pv2
                    src_arr = cand_v if kk < 8 else cand_v2
                    nc.vector.scalar_tensor_tensor(
                        out=mprod[:], in0=src_arr[:],
                        scalar=pv[:, kk % 8 : kk % 8 + 1], in1=cand_i[:],
                        op0=mybir.AluOpType.is_equal, op1=mybir.AluOpType.mult,
                        accum_out=pidx[:, kk : kk + 1])

                pidx_u = small.tile([128, LP], U32, tag="pidx_u")
                nc.vector.tensor_copy(pidx_u[:], pidx[:])

                G = gat.tile([128, LP * D], F32, tag="G")
                GV = G[:].rearrange("p (c w) -> p c w", c=LP)
                for c in range(LP):
                    nc.gpsimd.indirect_dma_start(
                        out=G[:, c * D : (c + 1) * D],
                        out_offset=None,
                        in_=mem_in[:],
                        in_offset=IndirectOffsetOnAxis(ap=pidx_u[:, c : c + 1], axis=0))

                nqf = small.tile([128, D], F32, tag="nqf")
                nc.scalar.activation(nqf[:], qf[:], mybir.ActivationFunctionType.Copy, scale=qrin[:])
                refined = small.tile([128, LP], F32, tag="refined")
                H = RH
                prod = res_pool.tile([128, H * D], F32, tag="prod")
                nqb3 = nqf[:].rearrange("p (c d) -> p c d", c=1).to_broadcast([128, H, D])
                gn2 = small.tile([128, LP], F32, tag="gn2")
                for h in range(LP // RH):
                    g3 = GV[:, h * H:(h + 1) * H, 0:D]
                    nc.vector.tensor_tensor(
                        out=prod[:].rearrange("p (c d) -> p c d", c=H),
                        in0=g3, in1=nqb3, op=mybir.AluOpType.mult)
                    nc.vector.tensor_reduce(
                        out=refined[:, h * H : (h + 1) * H],
                        in_=prod[:].rearrange("p (c d) -> p c d", c=H),
                        axis=mybir.AxisListType.X, op=mybir.AluOpType.add)
                    nc.vector.tensor_tensor(
                        out=prod[:].rearrange("p (c d) -> p c d", c=H),
                        in0=g3, in1=g3, op=mybir.AluOpType.mult)
                    nc.vector.tensor_reduce(
                        out=gn2[:, h * H : (h + 1) * H],
                        in_=prod[:].rearrange("p (c d) -> p c d", c=H),
                        axis=mybir.AxisListType.X, op=mybir.AluOpType.add)
                gn = small.tile([128, LP], F32, tag="gn")
                nc.scalar.activation(gn[:], gn2[:], mybir.ActivationFunctionType.Sqrt)
                nc.vector.tensor_scalar_max(gn[:], gn[:], 1e-12)
                grin = small.tile([128, LP], F32, tag="grin")
                nc.vector.reciprocal(grin[:], gn[:])
                nc.vector.tensor_tensor(out=refined[:], in0=refined[:], in1=grin[:], op=mybir.AluOpType.mult)

                selfr = small.tile([128, LP], F32, tag="selfr")
                nc.vector.tensor_scalar(
                    out=selfr[:], in0=refined[:], scalar1=SELF_MATCH, scalar2=None,
                    op0=mybir.AluOpType.is_ge)
                nc.vector.tensor_scalar(
                    out=selfr[:], in0=selfr[:], scalar1=NEG, scalar2=None,
                    op0=mybir.AluOpType.mult)
                nc.vector.tensor_add(refined[:], refined[:], selfr[:])

                top8 = small.tile([128, 8], F32, tag="top8")
                nc.vector.max(out=top8[:], in_=refined[:])
                wmask = small.tile([128, LP], F32, tag="wmask")
                nc.vector.tensor_scalar(
                    out=wmask[:], in0=refined[:], scalar1=top8[:, 7:8],
                    scalar2=None, op0=mybir.AluOpType.is_ge)
                shift = small.tile([128, LP], F32, tag="shift")
                nc.vector.tensor_scalar(
                    out=shift[:], in0=refined[:], scalar1=top8[:, 0:1],
                    scalar2=None, op0=mybir.AluOpType.subtract)
                expv = small.tile([128, LP], F32, tag="expv")
                nc.scalar.activation(expv[:], shift[:], mybir.ActivationFunctionType.Exp)
                wts = small.tile([128, LP], F32, tag="wts")
                nc.vector.tensor_tensor(out=wts[:], in0=expv[:], in1=wmask[:], op=mybir.AluOpType.mult)
                zsum = small.tile([128, 1], F32, tag="zsum")
                nc.vector.tensor_reduce(out=zsum[:], in_=wts[:], axis=mybir.AxisListType.X, op=mybir.AluOpType.add)
                zrin = small.tile([128, 1], F32, tag="zrin")
                nc.vector.reciprocal(zrin[:], zsum[:])
                nc.vector.tensor_scalar(
                    out=wts[:], in0=wts[:], scalar1=zrin[:, 0:1], scalar2=None,
                    op0=mybir.AluOpType.mult)

                nc.vector.scalar_tensor_tensor(
                    out=GV[:, :, 0:D],
                    in0=GV[:, :, 0:D], scalar=1.0,
                    in1=wts[:].rearrange("p (c o) -> p c o", o=1)
                    .to_broadcast([128, LP, D]),
                    op0=mybir.AluOpType.mult, op1=mybir.AluOpType.mult)
                w = LP
                while w > 1:
                    nw = w // 2
                    nc.vector.tensor_add(GV[:, 0:nw, 0:D], GV[:, 0:nw, 0:D],
                                         GV[:, nw:w, 0:D])
                    w = nw
                acc = small.tile([128, D], F32, tag="acc")
                nc.vector.tensor_copy(acc[:], G[:, 0:D])

                asq = small.tile([128, D], F32, tag="sqtmp")
                an2 = small.tile([128, 1], F32, tag="an2")
                nc.scalar.activation(asq[:], acc[:], mybir.ActivationFunctionType.Square,
                                     accum_out=an2[:])
                an = small.tile([128, 1], F32, tag="an")
                nc.scalar.activation(an[:], an2[:], mybir.ActivationFunctionType.Sqrt)
                nc.vector.tensor_scalar_max(an[:], an[:], 1e-12)
                arin = small.tile([128, 1], F32, tag="arin")
                nc.vector.reciprocal(arin[:], an[:])
                scl = small.tile([128, 1], F32, tag="scl")
                nc.vector.tensor_tensor(out=scl[:], in0=arin[:], in1=qnorm[:], op=mybir.AluOpType.mult)
                ot = small.tile([128, D], F32, tag="ot")
                nc.scalar.activation(ot[:], acc[:], mybir.ActivationFunctionType.Copy, scale=scl[:])
                nc.sync.dma_start(out[q0 : q0 + 128, :], ot[:])

    return nc


_CACHED_NC = None


def _get_nc():
    global _CACHED_NC
    if _CACHED_NC is None:
        _install_patches()
        _CACHED_NC = _build()
    return _CACHED_NC


def kernel(query, memory, k):
    query = np.ascontiguousarray(np.asarray(query, dtype=np.float32))
    memory = np.ascontiguousarray(np.asarray(memory, dtype=np.float32))
    k_val = int(np.asarray(k))
    assert query.shape == (B, D) and memory.shape == (M, D), (query.shape, memory.shape)
    assert k_val == K, f"kernel compiled for k={K}, got {k_val}"

    from concourse.bass_utils import run_bass_kernel_spmd

    nc = _get_nc()
    in_maps = [
        {"q": query[i * B_LOC : (i + 1) * B_LOC], "mem": memory}
        for i in range(N_CORES)
    ]
    res = run_bass_kernel_spmd(nc, in_maps, list(range(N_CORES)))
    return np.concatenate([res.results[i]["out"] for i in range(N_CORES)], axis=0)

